# revision 6
# baseline (speedup 1.0000x reference)
"""Relative-position attention (TransformerXL-style) on 8 TRN2 NeuronCores.

Sharding: data-parallel over batch (b=8 -> 1 batch element per core); weights
replicated. No collectives needed.

Per-core pipeline (n=1024, dim=512, heads=8, d_head=64), v2:
  qT = Wq^T x^T, kT = Wk^T x^T   [inner, n]   (bf16 matmuls, fp32 psum)
  v  = x Wv                      [n, inner]
  per (128-row query tile m, head h), software-pipelined 2 deep:
    front(k):
      S_psum[128, 1024] = qTh_m^T kTh            (2 matmuls, one [128,1024] tile)
      T_psum[128, w]    = qTh_m^T relT[:, band]  (2 matmuls)
      t8 (fp8)  <- T_psum (copy, DVE/Pool/ACT) + clip-tail fills
      pos8[p, j] = t8[p, j + 127 - p]            (diagonal SBUF->SBUF DMA, fp8:
                                                  half the skew bytes of bf16)
      S_psum += pos8  (identity matmul, fp8 moving operand)
      P_sb (bf16), z = exp(0.125 * S_psum)       (single ACT op, fused row-sum)
      r = 1/z (DVE); pT = xbar-transpose(P_sb)   (InstDmaTransposeAnt, 14ns/tile)
    mid(k-1):
      av_psum[128, 64] += pT_jb^T-blocks @ v_jb_h   (8 matmuls, N=64 moving)
      o_att[m][:, 64h:] = av_psum * r            (normalization folded into the
                                                  required psum->sbuf copy)
      (h==7) toT = xbar-transpose(o_att[m])
    back(k-2, h==7):
      out_m = toT^T-blocks @ Wo + ones^T bo      (5 matmuls, K=1 bias trick)

The rel-pos table is host-preprocessed into relT[d, c] = rel_emb[1024 -
clip(c - 511, 0, 1024), d] so that pos_attn[i, j] = (q_i . relT[:, j - i +
1023]) and clipping is baked into the padded table.
"""
import sys

sys.path.insert(0, "/opt/trn_rl_repo")

import numpy as np

import concourse.bass as bass
import concourse.bacc as bacc
import concourse.mybir as mybir
import concourse.tile as tile
from concourse.ap import AP
from concourse.bass_utils import run_bass_kernel_spmd

F32 = mybir.dt.float32
BF16 = mybir.dt.bfloat16
FP8 = mybir.dt.float8e4

B, N, DIM = 8, 1024, 512
HEADS, DH = 8, 64
INNER = HEADS * DH
MAX_POS = 512
RELW = 2 * MAX_POS + 1        # 1025 rel-emb rows
RELTW = 2047                  # extended/clip-padded table width
TW = 1151                     # per-query-tile T width (1024 + 127)
TWPAD = 1152
KC = DIM // 128               # 4 contraction chunks
MT = N // 128                 # 8 query row tiles
SCALE = DH ** -0.5

_CACHE = {}

import os
CFG = {
    # engine for t8 copy, cycled per iteration: v=DVE, p=Pool, a=ACT
    "tcp": os.environ.get("K_TCP", "vvav"),
    # engine for o-head scale-copy: v=DVE, a=ACT
    "ocp": os.environ.get("K_OCP", "vvva"),
    "sb": int(os.environ.get("K_SB", "2")),     # s psum bufs
    "avb": int(os.environ.get("K_AVB", "2")),   # av/oproj psum bufs
    "t8b": int(os.environ.get("K_T8B", "3")),
    "pb": int(os.environ.get("K_PB", "5")),
    "ptb": int(os.environ.get("K_PTB", "6")),
    "psb": int(os.environ.get("K_PSB", "4")),
    "rb": int(os.environ.get("K_RB", "6")),
    "db": int(os.environ.get("K_DB", "3")),     # stage B offset
    "dc": int(os.environ.get("K_DC", "7")),     # stage C offset
    "dd": int(os.environ.get("K_DD", "9")),     # stage D offset
}


def _build_nc():
    nc = bacc.Bacc()
    xT_in = nc.declare_dram_parameter("xT", [DIM, N], BF16, isOutput=False)
    wq_in = nc.declare_dram_parameter("wq", [DIM, INNER], BF16, isOutput=False)
    wk_in = nc.declare_dram_parameter("wk", [DIM, INNER], BF16, isOutput=False)
    wv_in = nc.declare_dram_parameter("wv", [DIM, INNER], BF16, isOutput=False)
    wo_in = nc.declare_dram_parameter("wo", [INNER, DIM], BF16, isOutput=False)
    rel_in = nc.declare_dram_parameter("relT", [128, RELTW], BF16, isOutput=False)
    bo_in = nc.declare_dram_parameter("bo", [1, DIM], BF16, isOutput=False)
    id8_in = nc.declare_dram_parameter("id8", [128, 128], FP8, isOutput=False)
    out_ext = nc.declare_dram_parameter("out", [N, DIM], F32, isOutput=True)

    with tile.TileContext(nc) as tc:
        with tc.tile_pool(name="persist", bufs=1) as pp:
            # ---- load persistent operands ----
            xT_sb = [pp.tile([128, N], BF16, name=f"xT{k}") for k in range(KC)]
            wq_sb = [pp.tile([128, INNER], BF16, name=f"wq{k}") for k in range(KC)]
            wk_sb = [pp.tile([128, INNER], BF16, name=f"wk{k}") for k in range(KC)]
            wv_sb = [pp.tile([128, INNER], BF16, name=f"wv{k}") for k in range(KC)]
            wo_sb = [pp.tile([128, DIM], BF16, name=f"wo{k}") for k in range(KC)]
            rel_sb = pp.tile([128, RELTW], BF16)
            bo_sb = pp.tile([1, DIM], BF16)
            id8_sb = pp.tile([128, 128], FP8)
            ones_sb = pp.tile([1, 128], BF16)
            onesw_sb = pp.tile([128, 512], BF16)
            for k in range(KC):
                nc.sync.dma_start(out=xT_sb[k][:], in_=xT_in[128 * k:128 * (k + 1), :])
                nc.sync.dma_start(out=wq_sb[k][:], in_=wq_in[128 * k:128 * (k + 1), :])
                nc.sync.dma_start(out=wk_sb[k][:], in_=wk_in[128 * k:128 * (k + 1), :])
            for k in range(KC):
                nc.sync.dma_start(out=wv_sb[k][:], in_=wv_in[128 * k:128 * (k + 1), :])
            nc.sync.dma_start(out=rel_sb[:], in_=rel_in[:])
            nc.sync.dma_start(out=id8_sb[:], in_=id8_in[:])
            for k in range(KC):
                nc.sync.dma_start(out=wo_sb[k][:], in_=wo_in[128 * k:128 * (k + 1), :])
            nc.sync.dma_start(out=bo_sb[:], in_=bo_in[:])
            nc.gpsimd.memset(ones_sb[:], 1.0)
            nc.gpsimd.memset(onesw_sb[:], 1.0)

            # ---- projections ----
            qT_sb = [pp.tile([128, N], BF16, name=f"qT{t}") for t in range(KC)]
            kT_sb = [pp.tile([128, N], BF16, name=f"kT{t}") for t in range(KC)]
            v_sb = [pp.tile([128, INNER], BF16, name=f"v{t}") for t in range(MT)]
            o_att = [pp.tile([128, INNER], BF16, name=f"oatt{t}") for t in range(MT)]

            with tc.tile_pool(name="proj_ps", bufs=4, space="PSUM") as proj_ps:
                for t in range(KC):          # qT / kT tiles: inner rows 128t..
                    for jc in range(2):      # n column chunks of 512
                        for which, w_sb, dst in (("q", wq_sb, qT_sb), ("k", wk_sb, kT_sb)):
                            ps = proj_ps.tile([128, 512], F32, tag="pps",
                                              name=f"ps{which}{t}{jc}")
                            for k in range(KC):
                                nc.tensor.matmul(
                                    ps[:],
                                    w_sb[k][:, 128 * t:128 * (t + 1)],
                                    xT_sb[k][:, 512 * jc:512 * (jc + 1)],
                                    start=(k == 0), stop=(k == KC - 1))
                            nc.vector.tensor_copy(dst[t][:, 512 * jc:512 * (jc + 1)], ps[:])
                for t in range(MT):          # v tiles: n rows 128t..
                    ps = proj_ps.tile([128, 512], F32, tag="pps", name=f"psv{t}")
                    for k in range(KC):
                        nc.tensor.matmul(
                            ps[:],
                            xT_sb[k][:, 128 * t:128 * (t + 1)],
                            wv_sb[k][:],
                            start=(k == 0), stop=(k == KC - 1))
                    if t % 2 == 0:
                        nc.scalar.copy(v_sb[t][:], ps[:])
                    else:
                        nc.vector.tensor_copy(v_sb[t][:], ps[:])

            # ---- attention (software-pipelined 2 deep over k = m*8 + h) ----
            with tc.tile_pool(name="attn_sb", bufs=3) as asb, \
                 tc.tile_pool(name="s_ps", bufs=CFG["sb"], space="PSUM") as sps, \
                 tc.tile_pool(name="t_ps", bufs=1, space="PSUM") as tps, \
                 tc.tile_pool(name="av_ps", bufs=CFG["avb"], space="PSUM") as avps, \
                 tc.tile_pool(name="fin_sb", bufs=2) as osb:

                pos_st = {}   # k -> pos8 tile (skew launched 2 steps early)
                pt_st = {}    # k -> (r_sb, pT)
                to_st = {}    # m -> toT

                def stageA(k):
                    # rel-pos stream, runs 2 steps ahead of the S stream:
                    # T matmuls -> fp8 copy -> diagonal skew DMA
                    m, h = divmod(k, HEADS)
                    th, ph = h // 2, (h % 2) * 64
                    qh = qT_sb[th][ph:ph + 64, 128 * m:128 * (m + 1)]
                    off = 896 - 128 * m
                    lo = max(0, 128 * m - 385)
                    hi = min(1150, 128 * m + 639)
                    w = hi - lo + 1
                    t_ps = tps.tile([128, 1024], F32, name="t_ps")
                    for ci in range(2):
                        c0, cw = (0, 512) if ci == 0 else (512, w - 512)
                        nc.tensor.matmul(
                            t_ps[:, c0:c0 + cw],
                            qh,
                            rel_sb[ph:ph + 64, off + lo + c0:off + lo + c0 + cw],
                            start=True, stop=True)
                    t8 = asb.tile([128, TWPAD], FP8, name="t8", bufs=CFG["t8b"])
                    tce = CFG["tcp"][k % len(CFG["tcp"])]
                    if tce == "a":
                        nc.scalar.copy(t8[:, lo:lo + w], t_ps[:, 0:w])
                    else:
                        nc.vector.tensor_copy(t8[:, lo:lo + w], t_ps[:, 0:w])
                    if lo > 0:    # low clip tail: rows of rel_emb[1024]
                        nc.vector.tensor_scalar_mul(
                            t8[:, 0:lo], onesw_sb[:, 0:lo], t_ps[:, 0:1])
                    if hi < 1150:  # high clip tail: rows of rel_emb[0]
                        nc.vector.tensor_scalar_mul(
                            t8[:, hi + 1:1151], onesw_sb[:, 0:1150 - hi],
                            t_ps[:, w - 1:w])
                    # Toeplitz skew: pos8[p, j] = t8[p, j + 127 - p]
                    pos8 = asb.tile([128, N], FP8, name="pos8", bufs=CFG["pb"])
                    skew = AP(t8.tensor, t8.offset + 127,
                              [[TWPAD - 1, 128], [1, N]])
                    nc.sync.dma_start(out=pos8[:], in_=skew)
                    pos_st[k] = pos8

                def stageB(k):
                    # S matmuls + pos add (pos8 already landed) + exp + xbar PT
                    m, h = divmod(k, HEADS)
                    th, ph = h // 2, (h % 2) * 64
                    qh = qT_sb[th][ph:ph + 64, 128 * m:128 * (m + 1)]
                    pos8 = pos_st.pop(k)
                    s_ps = sps.tile([128, 1024], F32, name="s_ps")
                    for jc in range(2):
                        nc.tensor.matmul(
                            s_ps[:, 512 * jc:512 * (jc + 1)],
                            qh,
                            kT_sb[th][ph:ph + 64, 512 * jc:512 * (jc + 1)],
                            start=True, stop=False)
                    for jc in range(2):
                        nc.tensor.matmul(
                            s_ps[:, 512 * jc:512 * (jc + 1)],
                            id8_sb[:],
                            pos8[:, 512 * jc:512 * (jc + 1)],
                            start=False, stop=True)
                    # softmax (no max-subtraction: logits are O(5))
                    p_sb = asb.tile([128, N], BF16, name="p_sb", bufs=CFG["psb"])
                    z_sb = asb.tile([128, 1], F32, name="z_sb")
                    nc.scalar.activation(
                        p_sb[:], s_ps[:],
                        mybir.ActivationFunctionType.Exp,
                        scale=SCALE, accum_out=z_sb[:])
                    r_sb = asb.tile([128, 1], F32, name="r_sb", bufs=CFG["rb"])
                    nc.vector.reciprocal(r_sb[:], z_sb[:])
                    # P^T via DMA xbar transpose: pT[p, jb*128 + i] = P[i, jb*128 + p]
                    pT = asb.tile([128, N], BF16, name="pT", bufs=CFG["ptb"])
                    pt_out = AP(pT.tensor, pT.offset,
                                [[N, 128], [128, MT], [1, 128]])
                    nc.sync.dma_start_transpose(pt_out, p_sb[:])
                    pt_st[k] = (r_sb, pT)

                def stageC(k):
                    # attention @ V with normalization folded into the copy
                    m, h = divmod(k, HEADS)
                    r_sb, pT = pt_st.pop(k)
                    av = avps.tile([128, 512], F32, name="av_ps", tag="av")
                    for jb in range(MT):
                        nc.tensor.matmul(
                            av[:, 0:DH],
                            pT[:, 128 * jb:128 * (jb + 1)],
                            v_sb[jb][:, DH * h:DH * (h + 1)],
                            start=(jb == 0), stop=(jb == MT - 1))
                    oce = CFG["ocp"][k % len(CFG["ocp"])]
                    if oce == "a":
                        nc.scalar.mul(o_att[m][:, DH * h:DH * (h + 1)],
                                      av[:, 0:DH], r_sb[:])
                    else:
                        nc.vector.tensor_scalar_mul(
                            o_att[m][:, DH * h:DH * (h + 1)], av[:, 0:DH], r_sb[:])
                    if h == HEADS - 1:
                        toT = asb.tile([128, INNER], BF16, name="toT", bufs=3)
                        to_out = AP(toT.tensor, toT.offset,
                                    [[INNER, 128], [128, KC], [1, 128]])
                        nc.sync.dma_start_transpose(to_out, o_att[m][:])
                        to_st[m] = toT

                def stageD(k):
                    # output projection (toT landed 2 steps ago)
                    m, h = divmod(k, HEADS)
                    if h != HEADS - 1:
                        return
                    toT = to_st.pop(m)
                    o_ps = avps.tile([128, 512], F32, name="o_ps", tag="av")
                    for g in range(KC):
                        nc.tensor.matmul(
                            o_ps[:],
                            toT[:, 128 * g:128 * (g + 1)],
                            wo_sb[g][:],
                            start=(g == 0), stop=False)
                    nc.tensor.matmul(o_ps[:], ones_sb[:], bo_sb[:],
                                     start=False, stop=True)
                    o_sb = osb.tile([128, DIM], F32, name="o_sb")
                    if m % 2 == 0:
                        nc.scalar.copy(o_sb[:], o_ps[:])
                    else:
                        nc.vector.tensor_copy(o_sb[:], o_ps[:])
                    nc.sync.dma_start(
                        out=out_ext[128 * m:128 * (m + 1), :], in_=o_sb[:])

                NK = MT * HEADS
                DB, DC, DD = CFG["db"], CFG["dc"], CFG["dd"]
                for step in range(NK + DD + 1):
                    # stage order B,C,D,A within a step: the A-stream's
                    # t_ps-recycle wait lands at the tail of each engine's
                    # sequencer queue instead of blocking ready work.
                    if DB <= step < NK + DB:
                        stageB(step - DB)
                    if DC <= step < NK + DC:
                        stageC(step - DC)
                    if DD <= step < NK + DD:
                        stageD(step - DD)
                    if step < NK:
                        stageA(step)
    nc.compile()
    return nc


def _prep_inputs(x, Wq, Wkv, rel_emb, Wo, bo):
    import ml_dtypes
    tobf = lambda a: np.asarray(a, dtype=np.float32).astype(ml_dtypes.bfloat16)
    Wk = Wkv[:, :INNER]
    Wv = Wkv[:, INNER:]
    # relT[d, c] = rel_emb[1024 - clip(c - 511, 0, 1024), d], duplicated onto
    # partitions 64..127 so both head-parity quadrants can read it.
    c = np.arange(RELTW)
    rows = RELW - 1 - np.clip(c - (MAX_POS - 1), 0, RELW - 1)
    relT64 = np.ascontiguousarray(rel_emb[rows].T)          # [64, 2047]
    relT = np.concatenate([relT64, relT64], axis=0)         # [128, 2047]
    id8 = np.eye(128, dtype=np.float32).astype(ml_dtypes.float8_e4m3)
    base = {
        "wq": tobf(Wq), "wk": tobf(Wk), "wv": tobf(Wv), "wo": tobf(Wo),
        "relT": tobf(relT), "bo": tobf(bo.reshape(1, DIM)),
        "id8": id8,
    }
    in_maps = []
    for c_ in range(B):
        m = dict(base)
        m["xT"] = tobf(np.ascontiguousarray(x[c_].T))
        in_maps.append(m)
    return in_maps


def kernel(x, Wq, Wkv, rel_emb, Wo, bo):
    if "nc" not in _CACHE:
        _CACHE["nc"] = _build_nc()
    nc = _CACHE["nc"]
    in_maps = _prep_inputs(x, Wq, Wkv, rel_emb, Wo, bo)
    res = run_bass_kernel_spmd(nc, in_maps, list(range(B))).results
    out = np.stack([res[c]["out"] for c in range(B)]).astype(np.float32)
    return out


# revision 10
# speedup vs baseline: 1.0151x; 1.0151x over previous
"""Relative-position attention (TransformerXL-style) on 8 TRN2 NeuronCores.

Sharding: data-parallel over batch (b=8 -> 1 batch element per core); weights
replicated. No collectives needed.

Per-core pipeline (n=1024, dim=512, heads=8, d_head=64), v2:
  qT = Wq^T x^T, kT = Wk^T x^T   [inner, n]   (bf16 matmuls, fp32 psum)
  v  = x Wv                      [n, inner]
  per (128-row query tile m, head h), software-pipelined 2 deep:
    front(k):
      S_psum[128, 1024] = qTh_m^T kTh            (2 matmuls, one [128,1024] tile)
      T_psum[128, w]    = qTh_m^T relT[:, band]  (2 matmuls)
      t8 (fp8)  <- T_psum (copy, DVE/Pool/ACT) + clip-tail fills
      pos8[p, j] = t8[p, j + 127 - p]            (diagonal SBUF->SBUF DMA, fp8:
                                                  half the skew bytes of bf16)
      S_psum += pos8  (identity matmul, fp8 moving operand)
      P_sb (bf16), z = exp(0.125 * S_psum)       (single ACT op, fused row-sum)
      r = 1/z (DVE); pT = xbar-transpose(P_sb)   (InstDmaTransposeAnt, 14ns/tile)
    mid(k-1):
      av_psum[128, 64] += pT_jb^T-blocks @ v_jb_h   (8 matmuls, N=64 moving)
      o_att[m][:, 64h:] = av_psum * r            (normalization folded into the
                                                  required psum->sbuf copy)
      (h==7) toT = xbar-transpose(o_att[m])
    back(k-2, h==7):
      out_m = toT^T-blocks @ Wo + ones^T bo      (5 matmuls, K=1 bias trick)

The rel-pos table is host-preprocessed into relT[d, c] = rel_emb[1024 -
clip(c - 511, 0, 1024), d] so that pos_attn[i, j] = (q_i . relT[:, j - i +
1023]) and clipping is baked into the padded table.
"""
import sys

sys.path.insert(0, "/opt/trn_rl_repo")

import numpy as np

import concourse.bass as bass
import concourse.bacc as bacc
import concourse.mybir as mybir
import concourse.tile as tile
from concourse.ap import AP
from concourse.bass_utils import run_bass_kernel_spmd

F32 = mybir.dt.float32
BF16 = mybir.dt.bfloat16
FP8 = mybir.dt.float8e4

B, N, DIM = 8, 1024, 512
HEADS, DH = 8, 64
INNER = HEADS * DH
MAX_POS = 512
RELW = 2 * MAX_POS + 1        # 1025 rel-emb rows
RELTW = 2047                  # extended/clip-padded table width
TW = 1151                     # per-query-tile T width (1024 + 127)
TWPAD = 1152
KC = DIM // 128               # 4 contraction chunks
MT = N // 128                 # 8 query row tiles
SCALE = DH ** -0.5

_CACHE = {}

import os
CFG = {
    # engine for t8 copy, cycled per iteration: v=DVE, p=Pool, a=ACT
    "tcp": os.environ.get("K_TCP", "vavv"),
    # engine for o-head scale-copy: v=DVE, a=ACT
    "ocp": os.environ.get("K_OCP", "v"),
    "sb": int(os.environ.get("K_SB", "2")),     # s psum bufs
    "avb": int(os.environ.get("K_AVB", "2")),   # av/oproj psum bufs
    "t8b": int(os.environ.get("K_T8B", "3")),
    "pb": int(os.environ.get("K_PB", "5")),
    "ptb": int(os.environ.get("K_PTB", "6")),
    "psb": int(os.environ.get("K_PSB", "4")),
    "rb": int(os.environ.get("K_RB", "6")),
    "db": int(os.environ.get("K_DB", "3")),     # stage B offset
    "dc": int(os.environ.get("K_DC", "7")),     # stage C offset
    "dd": int(os.environ.get("K_DD", "9")),     # stage D offset
}


def _build_nc():
    nc = bacc.Bacc()
    xT_in = nc.declare_dram_parameter("xT", [DIM, N], BF16, isOutput=False)
    wq_in = nc.declare_dram_parameter("wq", [DIM, INNER], BF16, isOutput=False)
    wk_in = nc.declare_dram_parameter("wk", [DIM, INNER], BF16, isOutput=False)
    wv_in = nc.declare_dram_parameter("wv", [DIM, INNER], BF16, isOutput=False)
    wo_in = nc.declare_dram_parameter("wo", [INNER, DIM], BF16, isOutput=False)
    rel_in = nc.declare_dram_parameter("relT", [128, RELTW], BF16, isOutput=False)
    bo_in = nc.declare_dram_parameter("bo", [1, DIM], BF16, isOutput=False)
    id8_in = nc.declare_dram_parameter("id8", [128, 128], FP8, isOutput=False)
    out_ext = nc.declare_dram_parameter("out", [N, DIM], F32, isOutput=True)

    with tile.TileContext(nc) as tc:
        with tc.tile_pool(name="persist", bufs=1) as pp:
            # ---- load persistent operands ----
            xT_sb = [pp.tile([128, N], BF16, name=f"xT{k}") for k in range(KC)]
            wq_sb = [pp.tile([128, INNER], BF16, name=f"wq{k}") for k in range(KC)]
            wk_sb = [pp.tile([128, INNER], BF16, name=f"wk{k}") for k in range(KC)]
            wv_sb = [pp.tile([128, INNER], BF16, name=f"wv{k}") for k in range(KC)]
            wo_sb = [pp.tile([128, DIM], BF16, name=f"wo{k}") for k in range(KC)]
            rel_sb = pp.tile([128, RELTW], BF16)
            bo_sb = pp.tile([1, DIM], BF16)
            id8_sb = pp.tile([128, 128], FP8)
            ones_sb = pp.tile([1, 128], BF16)
            onesw_sb = pp.tile([128, 512], BF16)
            for k in range(KC):
                nc.sync.dma_start(out=xT_sb[k][:], in_=xT_in[128 * k:128 * (k + 1), :])
                nc.sync.dma_start(out=wq_sb[k][:], in_=wq_in[128 * k:128 * (k + 1), :])
                nc.sync.dma_start(out=wk_sb[k][:], in_=wk_in[128 * k:128 * (k + 1), :])
            for k in range(KC):
                nc.sync.dma_start(out=wv_sb[k][:], in_=wv_in[128 * k:128 * (k + 1), :])
            nc.sync.dma_start(out=rel_sb[:], in_=rel_in[:])
            nc.sync.dma_start(out=id8_sb[:], in_=id8_in[:])
            for k in range(KC):
                nc.sync.dma_start(out=wo_sb[k][:], in_=wo_in[128 * k:128 * (k + 1), :])
            nc.sync.dma_start(out=bo_sb[:], in_=bo_in[:])
            nc.gpsimd.memset(ones_sb[:], 1.0)
            nc.gpsimd.memset(onesw_sb[:], 1.0)

            # ---- projections ----
            qT_sb = [pp.tile([128, N], BF16, name=f"qT{t}") for t in range(KC)]
            kT_sb = [pp.tile([128, N], BF16, name=f"kT{t}") for t in range(KC)]
            v_sb = [pp.tile([128, INNER], BF16, name=f"v{t}") for t in range(MT)]
            o_att = [pp.tile([128, INNER], BF16, name=f"oatt{t}") for t in range(MT)]

            with tc.tile_pool(name="proj_ps", bufs=4, space="PSUM") as proj_ps:
                for t in range(KC):          # qT / kT tiles: inner rows 128t..
                    for jc in range(2):      # n column chunks of 512
                        for which, w_sb, dst in (("q", wq_sb, qT_sb), ("k", wk_sb, kT_sb)):
                            ps = proj_ps.tile([128, 512], F32, tag="pps",
                                              name=f"ps{which}{t}{jc}")
                            for k in range(KC):
                                nc.tensor.matmul(
                                    ps[:],
                                    w_sb[k][:, 128 * t:128 * (t + 1)],
                                    xT_sb[k][:, 512 * jc:512 * (jc + 1)],
                                    start=(k == 0), stop=(k == KC - 1))
                            nc.vector.tensor_copy(dst[t][:, 512 * jc:512 * (jc + 1)], ps[:])
                for t in range(MT):          # v tiles: n rows 128t..
                    ps = proj_ps.tile([128, 512], F32, tag="pps", name=f"psv{t}")
                    for k in range(KC):
                        nc.tensor.matmul(
                            ps[:],
                            xT_sb[k][:, 128 * t:128 * (t + 1)],
                            wv_sb[k][:],
                            start=(k == 0), stop=(k == KC - 1))
                    if t % 2 == 0:
                        nc.scalar.copy(v_sb[t][:], ps[:])
                    else:
                        nc.vector.tensor_copy(v_sb[t][:], ps[:])

            # ---- attention (software-pipelined 2 deep over k = m*8 + h) ----
            with tc.tile_pool(name="attn_sb", bufs=3) as asb, \
                 tc.tile_pool(name="s_ps", bufs=CFG["sb"], space="PSUM") as sps, \
                 tc.tile_pool(name="t_ps", bufs=1, space="PSUM") as tps, \
                 tc.tile_pool(name="av_ps", bufs=CFG["avb"], space="PSUM") as avps, \
                 tc.tile_pool(name="fin_sb", bufs=2) as osb:

                pos_st = {}   # k -> pos8 tile (skew launched 2 steps early)
                pt_st = {}    # k -> (r_sb, pT)
                to_st = {}    # m -> toT

                def stageA(k):
                    # rel-pos stream, runs 2 steps ahead of the S stream:
                    # T matmuls -> fp8 copy -> diagonal skew DMA
                    m, h = divmod(k, HEADS)
                    th, ph = h // 2, (h % 2) * 64
                    qh = qT_sb[th][ph:ph + 64, 128 * m:128 * (m + 1)]
                    off = 896 - 128 * m
                    lo = max(0, 128 * m - 385)
                    hi = min(1150, 128 * m + 639)
                    w = hi - lo + 1
                    t8 = asb.tile([128, TWPAD], FP8, name="t8", bufs=CFG["t8b"])
                    t_ps = tps.tile([128, 1024], F32, name="t_ps")
                    for ci in range(2):
                        c0, cw = (0, 512) if ci == 0 else (512, w - 512)
                        nc.tensor.matmul(
                            t_ps[:, c0:c0 + cw],
                            qh,
                            rel_sb[ph:ph + 64, off + lo + c0:off + lo + c0 + cw],
                            start=True, stop=True)
                        # two half-copies per step on different engines so the
                        # psum->sbuf drain is ~650ns, not 1.2us
                        tce = CFG["tcp"][(2 * k + ci) % len(CFG["tcp"])]
                        if tce == "a":
                            nc.scalar.copy(t8[:, lo + c0:lo + c0 + cw],
                                           t_ps[:, c0:c0 + cw])
                        else:
                            nc.vector.tensor_copy(t8[:, lo + c0:lo + c0 + cw],
                                                  t_ps[:, c0:c0 + cw])
                    if lo > 0:    # low clip tail: rows of rel_emb[1024]
                        nc.vector.tensor_scalar_mul(
                            t8[:, 0:lo], onesw_sb[:, 0:lo], t_ps[:, 0:1])
                    if hi < 1150:  # high clip tail: rows of rel_emb[0]
                        nc.vector.tensor_scalar_mul(
                            t8[:, hi + 1:1151], onesw_sb[:, 0:1150 - hi],
                            t_ps[:, w - 1:w])
                    # Toeplitz skew: pos8[p, j] = t8[p, j + 127 - p]
                    pos8 = asb.tile([128, N], FP8, name="pos8", bufs=CFG["pb"])
                    skew = AP(t8.tensor, t8.offset + 127,
                              [[TWPAD - 1, 128], [1, N]])
                    nc.sync.dma_start(out=pos8[:], in_=skew)
                    pos_st[k] = pos8

                def stageB(k):
                    # S matmuls + pos add (pos8 already landed) + exp + xbar PT
                    m, h = divmod(k, HEADS)
                    th, ph = h // 2, (h % 2) * 64
                    qh = qT_sb[th][ph:ph + 64, 128 * m:128 * (m + 1)]
                    pos8 = pos_st.pop(k)
                    s_ps = sps.tile([128, 1024], F32, name="s_ps")
                    for jc in range(2):
                        nc.tensor.matmul(
                            s_ps[:, 512 * jc:512 * (jc + 1)],
                            qh,
                            kT_sb[th][ph:ph + 64, 512 * jc:512 * (jc + 1)],
                            start=True, stop=False)
                    for jc in range(2):
                        nc.tensor.matmul(
                            s_ps[:, 512 * jc:512 * (jc + 1)],
                            id8_sb[:],
                            pos8[:, 512 * jc:512 * (jc + 1)],
                            start=False, stop=True)
                    # softmax (no max-subtraction: logits are O(5))
                    p_sb = asb.tile([128, N], BF16, name="p_sb", bufs=CFG["psb"])
                    z_sb = asb.tile([128, 1], F32, name="z_sb")
                    nc.scalar.activation(
                        p_sb[:], s_ps[:],
                        mybir.ActivationFunctionType.Exp,
                        scale=SCALE, accum_out=z_sb[:])
                    r_sb = asb.tile([128, 1], F32, name="r_sb", bufs=CFG["rb"])
                    nc.vector.reciprocal(r_sb[:], z_sb[:])
                    # P^T via DMA xbar transpose: pT[p, jb*128 + i] = P[i, jb*128 + p]
                    pT = asb.tile([128, N], BF16, name="pT", bufs=CFG["ptb"])
                    pt_out = AP(pT.tensor, pT.offset,
                                [[N, 128], [128, MT], [1, 128]])
                    nc.sync.dma_start_transpose(pt_out, p_sb[:])
                    pt_st[k] = (r_sb, pT)

                def stageC(k):
                    # attention @ V with normalization folded into the copy
                    m, h = divmod(k, HEADS)
                    r_sb, pT = pt_st.pop(k)
                    av = avps.tile([128, 512], F32, name="av_ps", tag="av")
                    for jb in range(MT):
                        nc.tensor.matmul(
                            av[:, 0:DH],
                            pT[:, 128 * jb:128 * (jb + 1)],
                            v_sb[jb][:, DH * h:DH * (h + 1)],
                            start=(jb == 0), stop=(jb == MT - 1))
                    oce = CFG["ocp"][k % len(CFG["ocp"])]
                    if oce == "a":
                        nc.scalar.mul(o_att[m][:, DH * h:DH * (h + 1)],
                                      av[:, 0:DH], r_sb[:])
                    else:
                        nc.vector.tensor_scalar_mul(
                            o_att[m][:, DH * h:DH * (h + 1)], av[:, 0:DH], r_sb[:])
                    if h == HEADS - 1:
                        toT = asb.tile([128, INNER], BF16, name="toT", bufs=3)
                        to_out = AP(toT.tensor, toT.offset,
                                    [[INNER, 128], [128, KC], [1, 128]])
                        nc.sync.dma_start_transpose(to_out, o_att[m][:])
                        to_st[m] = toT

                def stageD(k):
                    # output projection (toT landed 2 steps ago)
                    m, h = divmod(k, HEADS)
                    if h != HEADS - 1:
                        return
                    toT = to_st.pop(m)
                    o_ps = avps.tile([128, 512], F32, name="o_ps", tag="av")
                    for g in range(KC):
                        nc.tensor.matmul(
                            o_ps[:],
                            toT[:, 128 * g:128 * (g + 1)],
                            wo_sb[g][:],
                            start=(g == 0), stop=False)
                    nc.tensor.matmul(o_ps[:], ones_sb[:], bo_sb[:],
                                     start=False, stop=True)
                    o_sb = osb.tile([128, DIM], F32, name="o_sb")
                    if m % 2 == 0:
                        nc.scalar.copy(o_sb[:], o_ps[:])
                    else:
                        nc.vector.tensor_copy(o_sb[:], o_ps[:])
                    nc.sync.dma_start(
                        out=out_ext[128 * m:128 * (m + 1), :], in_=o_sb[:])

                NK = MT * HEADS
                DB, DC, DD = CFG["db"], CFG["dc"], CFG["dd"]
                for step in range(NK + DD + 1):
                    # stage order B,C,D,A within a step: the A-stream's
                    # t_ps-recycle wait lands at the tail of each engine's
                    # sequencer queue instead of blocking ready work.
                    if DB <= step < NK + DB:
                        stageB(step - DB)
                    if DC <= step < NK + DC:
                        stageC(step - DC)
                    if DD <= step < NK + DD:
                        stageD(step - DD)
                    if step < NK:
                        stageA(step)
    nc.compile()
    return nc


def _prep_inputs(x, Wq, Wkv, rel_emb, Wo, bo):
    import ml_dtypes
    tobf = lambda a: np.asarray(a, dtype=np.float32).astype(ml_dtypes.bfloat16)
    Wk = Wkv[:, :INNER]
    Wv = Wkv[:, INNER:]
    # relT[d, c] = rel_emb[1024 - clip(c - 511, 0, 1024), d], duplicated onto
    # partitions 64..127 so both head-parity quadrants can read it.
    c = np.arange(RELTW)
    rows = RELW - 1 - np.clip(c - (MAX_POS - 1), 0, RELW - 1)
    relT64 = np.ascontiguousarray(rel_emb[rows].T)          # [64, 2047]
    relT = np.concatenate([relT64, relT64], axis=0)         # [128, 2047]
    id8 = np.eye(128, dtype=np.float32).astype(ml_dtypes.float8_e4m3)
    base = {
        "wq": tobf(Wq), "wk": tobf(Wk), "wv": tobf(Wv), "wo": tobf(Wo),
        "relT": tobf(relT), "bo": tobf(bo.reshape(1, DIM)),
        "id8": id8,
    }
    in_maps = []
    for c_ in range(B):
        m = dict(base)
        m["xT"] = tobf(np.ascontiguousarray(x[c_].T))
        in_maps.append(m)
    return in_maps


def kernel(x, Wq, Wkv, rel_emb, Wo, bo):
    if "nc" not in _CACHE:
        _CACHE["nc"] = _build_nc()
    nc = _CACHE["nc"]
    in_maps = _prep_inputs(x, Wq, Wkv, rel_emb, Wo, bo)
    res = run_bass_kernel_spmd(nc, in_maps, list(range(B))).results
    out = np.stack([res[c]["out"] for c in range(B)]).astype(np.float32)
    return out
